# revision 1
# baseline (speedup 1.0000x reference)
"""DIN attention Bass kernel for Trainium2, 8-core data-parallel.

Math (per batch element b, with q = querys[b,0,:], K = keys[b]):
  x @ W1 + b1 = K @ (W1b-W1c) + (q*K) @ W1d + qb[b],
  qb = q @ (W1a+W1c) + b1 (host-precomputed, applied as relu1 bias)
  h1 = relu(.); h2 = relu(h1 @ W2 + b2); score = h2 @ W3 + b3
  u = exp(score + madd)  (madd = -1e4 where mask==0 plus b3; scores are
  O(1) so no max-subtraction); w = u/sum(u); out = sum_s w * K.

Per-core layouts (BL = 256 batch rows, chunks of CHUNK=32):
  keys_fm [128=E, b, s(pad 256)] bf16 feature-major, MLP moving operand
  keys_tp "paired" [s(<128), b, 2, 128=E] bf16 token-major: h-half 0 is
  row s, h-half 1 is row s+128 (h=1 rows >=72 zero) -> 512B descriptors.
  MLP in groups of 2 b (N=400), bf16 weights, fp32 PSUM.
  L2 packs 2 groups per PSUM bank at partition offsets 0/64 (col groups).
  L3 (lhsT [128,2] block-diag W3) places 4 pairs per bank at partitions
  {32j,32j+1}; a second accumulating matmul (lhsT = 2x2 identity, rhs =
  host-packed madd pair rows) adds the mask bias in PSUM.  exp runs as
  ONE [98,400] op straight off the sparse PSUM rows (dead rows produce
  garbage that is never read), softmax stats are [98,2] segmented ops,
  and w transposes to wcol via PE into a single PSUM bank, evacuated by
  one strided-AP copy selecting the 8 live columns per quadrant.
  ps3 row 32j+r, col-half c holds b = 16*bank + 4j + 2r + c.
  Weighted sum: out_col[b] = ktm[b].T @ wcol (K=s, M=128, N=1) into one
  PSUM bank [128, 32]; PE-transpose -> [32, 128] -> DMA out.
"""

import numpy as np

B, S, E = 2048, 200, 128
SP = 256                # padded S for clean 512B DMA descriptors
H1, H2 = 128, 64
NCORES = 8
BL = B // NCORES        # 256
CHUNK = 32
NCHUNK = BL // CHUNK    # 8
NEG = -10000.0

_prog = None


def _build_program():
    import concourse.bass as bass
    import concourse.mybir as mybir
    import concourse.tile as tile
    from concourse import bacc
    from concourse.masks import make_identity
    from contextlib import ExitStack

    f32 = mybir.dt.float32
    bf16 = mybir.dt.bfloat16
    AF = mybir.ActivationFunctionType
    ALU = mybir.AluOpType

    nc = bacc.Bacc(None, target_bir_lowering=False, debug=False)

    keys_fm = nc.declare_dram_parameter("keys_fm", [BL // CHUNK, E, CHUNK, S], bf16, False)
    keys_t1 = nc.declare_dram_parameter("keys_t1", [BL // CHUNK, 128, CHUNK, E], bf16, False)
    keys_t2 = nc.declare_dram_parameter("keys_t2", [BL // CHUNK, 72, CHUNK, E], bf16, False)
    qfm_d = nc.declare_dram_parameter("qfm", [E, BL], f32, False)
    qbf_d = nc.declare_dram_parameter("qbf", [H1, BL], f32, False)
    mskt_d = nc.declare_dram_parameter("mskt", [BL // CHUNK, 128, 64], bf16, False)
    b3v_d = nc.declare_dram_parameter("b3v", [1, 1], f32, False)
    w1bc_d = nc.declare_dram_parameter("w1bc", [E, H1], bf16, False)
    w1d_d = nc.declare_dram_parameter("w1d", [E, H1], bf16, False)
    w2_d = nc.declare_dram_parameter("w2", [H1, H2], bf16, False)
    w3s_d = nc.declare_dram_parameter("w3s", [2 * H2, 32], bf16, False)
    b2s_d = nc.declare_dram_parameter("b2s", [2 * H2, 1], f32, False)
    out_d = nc.declare_dram_parameter("out", [E, BL], f32, True)

    with tile.TileContext(nc) as tc, ExitStack() as ctx:
        const = ctx.enter_context(tc.tile_pool(name="const", bufs=1))
        kpool = ctx.enter_context(tc.tile_pool(name="keys", bufs=5))
        work = ctx.enter_context(tc.tile_pool(name="work", bufs=8))
        spool = ctx.enter_context(tc.tile_pool(name="smax", bufs=5))
        ps1p = ctx.enter_context(tc.tile_pool(name="ps1", bufs=4, space="PSUM"))
        ps2p = ctx.enter_context(tc.tile_pool(name="ps2", bufs=2, space="PSUM"))
        ps3p = ctx.enter_context(tc.tile_pool(name="ps3", bufs=1, space="PSUM"))
        psop = ctx.enter_context(tc.tile_pool(name="pso", bufs=1, space="PSUM"))

        w1bc = const.tile([E, H1], bf16)
        nc.scalar.dma_start(w1bc, w1bc_d[:])
        w1d = const.tile([E, H1], bf16)
        nc.scalar.dma_start(w1d, w1d_d[:])
        w2 = const.tile([H1, H2], bf16)
        nc.scalar.dma_start(w2, w2_d[:])
        w3s = const.tile([2 * H2, 32], bf16)
        nc.scalar.dma_start(w3s, w3s_d[:])
        b2s = const.tile([2 * H2, 1], f32)
        nc.scalar.dma_start(b2s, b2s_d[:])
        qfm = const.tile([E, BL], f32)
        nc.scalar.dma_start(qfm, qfm_d[:])
        qbf = const.tile([H1, BL], f32)
        nc.scalar.dma_start(qbf, qbf_d[:])
        ident_bf = const.tile([128, 128], bf16)
        make_identity(nc, ident_bf)
        ones_c = const.tile([128, 1], bf16)
        nc.vector.memset(ones_c, 1.0)
        ones_r = const.tile([1, 128], f32)
        nc.vector.memset(ones_r, 1.0)
        b3t = const.tile([128, 1], f32)
        nc.scalar.dma_start(b3t, b3v_d[:].to_broadcast((128, 1)))
        mskt_all = const.tile([128, NCHUNK, 64], bf16)
        nc.scalar.dma_start(mskt_all, mskt_d[:].rearrange("c p x -> p c x"))

        for ch in range(NCHUNK):
            b0 = ch * CHUNK
            # ---- chunk input DMAs (one each, on the SP queue) ----
            kfm = kpool.tile([E, CHUNK, S], bf16, tag="kfm")
            for hh in range(2):
                nc.sync.dma_start(
                    kfm[:, hh * 16:(hh + 1) * 16, :],
                    keys_fm[ch, :, hh * 16:(hh + 1) * 16, :])
            # ktm1[s, b, e] = keys[b,s,:] (s<128); ktm2[s,b,e] = keys[b,s+128,:]
            ktm1 = kpool.tile([128, CHUNK, E], bf16, tag="ktm1")
            ktm2 = kpool.tile([72, CHUNK, E], bf16, tag="ktm2")
            for hh in range(2):
                nc.sync.dma_start(
                    ktm1[:, hh * 16:(hh + 1) * 16, :],
                    keys_t1[ch, :, hh * 16:(hh + 1) * 16, :])
            nc.sync.dma_start(ktm2, keys_t2[ch])
            # transposed mask, packed (loaded once up front)
            mskt = mskt_all[:, ch, :]

            # ---- MLP over 16 groups of 2 batch rows ----
            pso = psop.tile([128, 512], f32, tag="o")
            ups = pso[:, 448:480]  # masked sums U at partition 0, cols 448+
            ps2 = None
            ps3 = None
            for g in range(CHUNK // 2):
                qk = work.tile([E, 2, S], bf16, tag="qk")
                for i in range(2):
                    k = 2 * g + i
                    nc.gpsimd.tensor_scalar(
                        qk[:, i, :], kfm[:, k, :],
                        qfm[:, b0 + k:b0 + k + 1], None, ALU.mult,
                    )
                ps1 = ps1p.tile([E, 2, S], f32, tag="ps1")
                nc.tensor.matmul(ps1, w1bc,
                                 kfm[:, 2 * g:2 * g + 2, :],
                                 start=True, stop=False, skip_group_check=True)
                nc.tensor.matmul(ps1, w1d, qk, start=False, stop=True,
                                 skip_group_check=True)
                h1 = work.tile([H1, 2, S], bf16, tag="h1")
                for i in range(2):
                    k = 2 * g + i
                    qb_col = qbf[:, b0 + k:b0 + k + 1]
                    if (g + i) % 2 == 1 and not (g == 0 and i == 1):
                        nc.scalar.activation(h1[:, i, :], ps1[:, i, :],
                                             AF.Relu, bias=qb_col)
                    else:
                        nc.vector.tensor_scalar(h1[:, i, :], ps1[:, i, :],
                                                qb_col, 0.0, ALU.add, ALU.max)

                if g % 2 == 0:
                    ps2 = ps2p.tile([128, 2 * S], f32, tag="ps2")
                off = 64 * (g % 2)
                nc.tensor.matmul(
                    ps2[off:off + H2, :], w2,
                    h1.rearrange("p a b -> p (a b)"),
                    start=True, stop=True, tile_position=(0, off),
                    skip_group_check=True,
                )
                if g % 2 == 1:
                    h2 = work.tile([128, 2 * S], bf16, tag="h2")
                    if g % 4 == 1 or g % 4 == 3 and ch % 2 == 0:
                        nc.scalar.activation(h2, ps2, AF.Relu, bias=b2s[:, 0:1])
                    else:
                        nc.vector.tensor_scalar(h2, ps2, b2s[:, 0:1], 0.0,
                                                ALU.add, ALU.max)
                    p = g // 2  # pair index 0..7; covers b = 4p..4p+3
                    j = p % 4
                    if j == 0:
                        ps3 = ps3p.tile([128, 2 * S], f32, tag="ps3")
                    nc.tensor.matmul(ps3[32 * j:32 * j + 32, :], w3s, h2,
                                     start=True, stop=True,
                                     tile_position=(0, 32 * j),
                                     skip_group_check=True)
                    if j == 3:
                        bank = p // 4  # 0 or 1 within chunk
                        # ---- unnormalized u = exp(score + b3) ----
                        u_sp = spool.tile([98, 2 * S], bf16, tag="usp")
                        nc.scalar.activation(u_sp, ps3[0:98, :], AF.Exp,
                                             bias=b3t[0:98, 0:1])
                        u3 = u_sp.rearrange("p (c s) -> p c s", s=S)
                        # ---- transpose u quadrants into one PSUM bank ----
                        # quadrant q = 2c+hh: wps[:, q, y] = u[b(row y), s-half hh]
                        wps = ps3p.tile([128, 4, 128], bf16, tag="ps3")
                        for c in range(2):
                            nc.tensor.transpose(
                                wps[:, 2 * c, 0:98], u3[:, c, 0:128],
                                ident_bf[0:98, 0:98])
                            nc.tensor.transpose(
                                wps[0:72, 2 * c + 1, 0:98], u3[:, c, 128:200],
                                ident_bf[0:98, 0:98])
                        # live columns y = 32j + r; mask applied during evac
                        wv = wps.rearrange("p q (j x) -> p q j x", x=32)
                        mv = mskt.rearrange(
                            "p (bk h c j r) -> p bk h c j r", bk=2, h=2,
                            c=2, j=4)
                        wcolA = spool.tile([128, 2, 4, 2], bf16,
                                           tag=f"wcolA{bank}")
                        nc.vector.tensor_tensor(wcolA, wv[:, 0:4:2, :, 0:2],
                                                mv[:, bank, 0], ALU.mult)
                        wcolB = spool.tile([72, 2, 4, 2], bf16,
                                           tag=f"wcolB{bank}")
                        nc.vector.tensor_tensor(wcolB, wv[0:72, 1:4:2, :, 0:2],
                                                mv[0:72, bank, 1], ALU.mult)
                        # masked sums U[b] via ones-contraction on PE
                        nc.tensor.matmul(
                            ups[0:1, 16 * bank:16 * bank + 16], ones_c,
                            wcolA.rearrange("p a b c -> p b c a"),
                            start=True, stop=False, skip_group_check=True)
                        nc.tensor.matmul(
                            ups[0:1, 16 * bank:16 * bank + 16], ones_c[0:72, :],
                            wcolB.rearrange("p a b c -> p b c a"),
                            start=False, stop=True, skip_group_check=True)
                        if bank == 0:
                            wcol_b0 = (wcolA, wcolB)
                        else:
                            wcol_b1 = (wcolA, wcolB)

            # ---- weighted sum: out_col[:,k] = ktm[k].T @ wcol ----
            for k in range(CHUNK):
                wcA, wcB = wcol_b0 if k < 16 else wcol_b1
                j, r, c = (k % 16) // 4, (k % 4) // 2, k % 2
                nc.tensor.matmul(pso[:, k:k + 1], ktm1[:, k, :],
                                 wcA[:, c, j, r:r + 1],
                                 start=True, stop=False, skip_group_check=True)
                nc.tensor.matmul(pso[:, k:k + 1], ktm2[:, k, :],
                                 wcB[:, c, j, r:r + 1],
                                 start=False, stop=True, skip_group_check=True)
            uinv = spool.tile([1, CHUNK], f32, tag="uinv")
            nc.vector.reciprocal(uinv, ups[0:1, 0:CHUNK])
            # broadcast 1/U to all partitions: ones[1,128]^T @ uinv[1,32]
            nc.tensor.matmul(pso[:, 480:512], ones_r, uinv,
                             start=True, stop=True, skip_group_check=True)
            uinvb = spool.tile([128, CHUNK], f32, tag="uinvb")
            nc.scalar.activation(uinvb, pso[:, 480:512], AF.Copy)
            ost = work.tile([128, CHUNK], f32, tag="ost")
            nc.vector.tensor_tensor(ost, pso[:, 0:CHUNK], uinvb, ALU.mult)
            if ch == NCHUNK - 1:
                nc.scalar.dma_start(out_d[:, b0:b0 + CHUNK], ost)
            else:
                nc.gpsimd.dma_start(out_d[:, b0:b0 + CHUNK], ost)
    nc.compile()
    return nc


def _host_prep(querys, keys, W1, b1, W2, b2, W3, b3, mask):
    import ml_dtypes
    bf = ml_dtypes.bfloat16
    q = np.ascontiguousarray(querys[:, 0, :], dtype=np.float32)       # [B,E]
    W1a, W1b, W1c, W1d = W1[0:128], W1[128:256], W1[256:384], W1[384:512]
    W1bc = (W1b - W1c).astype(bf)
    qb = q @ (W1a + W1c) + b1                                          # [B,H1]
    qbf = np.ascontiguousarray(qb.T, dtype=np.float32)                 # [H1,B]
    qfm = np.ascontiguousarray(q.T)                                    # [E,B]
    kbf = keys.astype(bf)
    # pre-chunked layouts: multi-KB contiguous DMA descriptors, no padding
    kfm_c = np.ascontiguousarray(
        kbf.transpose(0, 2, 1).reshape(B // CHUNK, CHUNK, E, S)
        .transpose(0, 2, 1, 3))                       # [B/32, E, 32, S]
    kt = kbf.reshape(B // CHUNK, CHUNK, S, E).transpose(0, 2, 1, 3)
    ktm1_c = np.ascontiguousarray(kt[:, 0:128])       # [B/32, 128, 32, E]
    ktm2_c = np.ascontiguousarray(kt[:, 128:200])     # [B/32, 72, 32, E]
    # transposed mask packed to match the wcol evac APs:
    # mskt[chunk, s-row, bank, h, c, j, r]; h=0 covers s<128, h=1 s>=128
    m3 = mask.astype(np.float32).reshape(B // CHUNK, CHUNK, S)
    mskt = np.zeros((B // CHUNK, 128, 2, 2, 2, 4, 2), np.float32)
    for bank in range(2):
        for c in range(2):
            for j in range(4):
                for r in range(2):
                    bloc = 16 * bank + 4 * j + 2 * r + c
                    mskt[:, :, bank, 0, c, j, r] = m3[:, bloc, 0:128]
                    mskt[:, 0:72, bank, 1, c, j, r] = m3[:, bloc, 128:200]
    mskt = np.ascontiguousarray(mskt.reshape(B // CHUNK, 128, 64)).astype(bf)
    w3s = np.zeros((2 * H2, 32), bf)
    w3s[0:H2, 0] = W3[:, 0].astype(bf)
    w3s[H2:, 1] = W3[:, 0].astype(bf)
    b2s = np.concatenate([b2, b2]).reshape(2 * H2, 1).astype(np.float32)
    return dict(W1bc=W1bc, W1d=W1d.astype(bf), qfm=qfm, qbf=qbf,
                kfm_c=kfm_c, ktm1_c=ktm1_c, ktm2_c=ktm2_c, mskt=mskt,
                b3v=np.asarray(b3, np.float32).reshape(1, 1),
                w3s=w3s, b2s=b2s, W2=W2.astype(bf))


def kernel(querys, keys, W1, b1, W2, b2, W3, b3, mask):
    global _prog
    from concourse.bass_utils import run_bass_kernel_spmd

    querys = np.asarray(querys, dtype=np.float32)
    keys = np.asarray(keys, dtype=np.float32)
    W1 = np.asarray(W1, dtype=np.float32)
    b1 = np.asarray(b1, dtype=np.float32)
    W2 = np.asarray(W2, dtype=np.float32)
    b2 = np.asarray(b2, dtype=np.float32)
    W3 = np.asarray(W3, dtype=np.float32)
    b3 = np.asarray(b3, dtype=np.float32)
    mask = np.asarray(mask)
    hp = _host_prep(querys, keys, W1, b1, W2, b2, W3, b3, mask)

    if _prog is None:
        _prog = _build_program()

    in_maps = []
    for core in range(NCORES):
        sl = slice(core * BL, (core + 1) * BL)
        slc = slice(core * BL // CHUNK, (core + 1) * BL // CHUNK)
        in_maps.append({
            "keys_fm": hp["kfm_c"][slc],
            "keys_t1": hp["ktm1_c"][slc],
            "keys_t2": hp["ktm2_c"][slc],
            "qfm": np.ascontiguousarray(hp["qfm"][:, sl]),
            "qbf": np.ascontiguousarray(hp["qbf"][:, sl]),
            "mskt": hp["mskt"][slc],
            "b3v": hp["b3v"],
            "w1bc": hp["W1bc"],
            "w1d": hp["W1d"],
            "w2": hp["W2"],
            "w3s": hp["w3s"],
            "b2s": hp["b2s"],
        })

    res = run_bass_kernel_spmd(_prog, in_maps, list(range(NCORES)))
    out = np.empty((B, E), np.float32)
    for core in range(NCORES):
        out[core * BL:(core + 1) * BL] = res.results[core]["out"].T
    return out



# revision 10
# speedup vs baseline: 2.0284x; 2.0284x over previous
"""DIN attention Bass kernel for Trainium2, 8-core data-parallel.

Design (per core, BL=256 rows, 8 chunks of 32 rows):
- Token compaction: only unmasked tokens are shipped (max count 127 <= 128).
  Rows are globally sorted by token count into 8 bands of 256; band ci is
  chunk index ci on every core, so all cores share per-chunk width W_c
  (multiple of 8), keeping the SPMD program identical across cores.
- W1 fused as ONE fp8 DoubleRow matmul per row: plane0 = K + a_b,
  plane1 = q_b*K + v_b where [a_b; v_b] is the min-norm solution of
  [W1bc.T | W1d.T] [a; v] = qb (qb = q(W1a+W1c)+b1).  The relu1 bias
  vanishes; relu1 becomes a big batched op.
- Layout per chunk: 4 ps1 tiles (2 PSUM banks each) hold 8 rows' h1-pre;
  relu1 -> h1 [128, 8, W] bf16.  W2 packs 8 rows per ps2 bank
  (2 partition-halves x 4 col-quarters); relu2 [128, 2, 4W] -> h2.
  W3 (block-diag w3s [128, 2]) -> scores at ps3 rows {32j, 32j+1}.
  exp as one [98, 4W] op; 4 PE transposes -> wps [W, 4, 98] bf16.
- Weighted sum: per-row N=1 matmul (ktm stationary, free ldweights);
  U via per-row mask-column stationary (N=1).  sums+U DMA'd out
  unnormalized; host divides (and unsorts).
"""

import numpy as np

B, S, E = 2048, 200, 128
H1, H2 = 128, 64
NCORES = 8
BL = B // NCORES          # 256
CHUNK = 32
NCHUNK = BL // CHUNK      # 8

_prog_cache = {}


def _build_program(widths):
    import concourse.bass as bass
    import concourse.mybir as mybir
    import concourse.tile as tile
    from concourse import bacc
    from concourse.masks import make_identity
    from contextlib import ExitStack

    f32 = mybir.dt.float32
    bf16 = mybir.dt.bfloat16
    fp8 = mybir.dt.float8e4
    AF = mybir.ActivationFunctionType
    ALU = mybir.AluOpType
    DR = mybir.MatmulPerfMode.DoubleRow

    nc = bacc.Bacc(None, target_bir_lowering=False, debug=False)

    rhs_d = [nc.declare_dram_parameter(f"rhs{ci}", [E, CHUNK, 2, widths[ci]],
                                       fp8, False) for ci in range(NCHUNK)]
    ktm_d = [nc.declare_dram_parameter(f"ktm{ci}", [widths[ci], CHUNK, E],
                                       bf16, False) for ci in range(NCHUNK)]
    mskt_d = nc.declare_dram_parameter("mskt", [128, NCHUNK, CHUNK], bf16, False)
    w1dr_d = nc.declare_dram_parameter("w1dr", [E, 2, H1], fp8, False)
    w2_d = nc.declare_dram_parameter("w2", [H1, H2], bf16, False)
    w3s_d = nc.declare_dram_parameter("w3s", [2 * H2, 32], bf16, False)
    b2s_d = nc.declare_dram_parameter("b2s", [2 * H2, 1], f32, False)
    b3v_d = nc.declare_dram_parameter("b3v", [1, 1], f32, False)
    out_d = nc.declare_dram_parameter("out", [E, NCHUNK * 2 * CHUNK], f32, True)

    with tile.TileContext(nc) as tc, ExitStack() as ctx:
        const = ctx.enter_context(tc.tile_pool(name="const", bufs=1))
        kpool = ctx.enter_context(tc.tile_pool(name="keys", bufs=1))
        work = ctx.enter_context(tc.tile_pool(name="work", bufs=3))
        spool = ctx.enter_context(tc.tile_pool(name="smax", bufs=2))
        ps1p = ctx.enter_context(tc.tile_pool(name="ps1", bufs=2, space="PSUM"))
        ps2p = ctx.enter_context(tc.tile_pool(name="ps2", bufs=1, space="PSUM"))
        ps3p = ctx.enter_context(tc.tile_pool(name="ps3", bufs=1, space="PSUM"))
        wpsp = ctx.enter_context(tc.tile_pool(name="wps", bufs=1, space="PSUM"))

        w1dr = const.tile([E, 2, H1], fp8)
        nc.scalar.dma_start(w1dr, w1dr_d[:])
        w2c = const.tile([H1, H2], bf16)
        nc.scalar.dma_start(w2c, w2_d[:])
        w3s = const.tile([2 * H2, 32], bf16)
        nc.scalar.dma_start(w3s, w3s_d[:])
        b2s = const.tile([2 * H2, 1], f32)
        nc.scalar.dma_start(b2s, b2s_d[:])
        b3t = const.tile([128, 1], f32)
        nc.scalar.dma_start(b3t, b3v_d[:].to_broadcast((128, 1)))
        mskt = const.tile([128, NCHUNK, CHUNK], bf16)
        nc.scalar.dma_start(mskt, mskt_d[:])
        ident_bf = const.tile([128, 128], bf16)
        make_identity(nc, ident_bf)
        outbuf = const.tile([E, NCHUNK, 2 * CHUNK], f32)
        nc.vector.memset(outbuf, 0.0)

        for ci in range(NCHUNK):
            W = widths[ci]
            # ---- input DMAs: alternate queues per chunk for balance ----
            rhs8 = kpool.tile([E, CHUNK, 2, W], fp8, tag=f"rhs{ci}")
            ktm = kpool.tile([128, CHUNK, E], bf16, tag=f"ktm{ci}")
            if ci % 2 == 0:
                nc.sync.dma_start(rhs8, rhs_d[ci][:])
                nc.gpsimd.dma_start(ktm[0:W], ktm_d[ci][:])
            else:
                nc.gpsimd.dma_start(rhs8, rhs_d[ci][:])
                nc.sync.dma_start(ktm[0:W], ktm_d[ci][:])

            ps3 = ps3p.tile([128, 512], f32, tag="ps3")
            pso = ps3[:, 448:512]
            h1s = []
            for t in range(4):        # 8 rows per t
                ps1 = ps1p.tile([128, 2, 512], f32, tag="ps1")
                for u in range(8):
                    slot = 8 * t + u
                    nc.tensor.matmul(
                        ps1[:, u // 4, (u % 4) * W:(u % 4 + 1) * W],
                        w1dr, rhs8[:, slot, :, :],
                        start=True, stop=True, perf_mode=DR,
                        skip_group_check=True)
                h1 = work.tile([128, 8, W], bf16, tag="h1")
                h1s.append(h1)
                # relu1: one [128, 2, 4, W] op (no bias)
                src = ps1[:, :, 0:4 * W].rearrange("p a (c b) -> p a c b", b=W)
                dst = h1.rearrange("p (a c) b -> p a c b", a=2)
                if t % 2 == 0:
                    nc.scalar.activation(dst, src, AF.Relu)
                else:
                    nc.vector.tensor_scalar(dst, src, 0.0, None, ALU.max)
                # W2: 8 rows into one ps2 bank-half
                if t % 2 == 0:
                    ps2 = ps2p.tile([128, 2, 512], f32, tag="ps2")
                for u in range(8):
                    qq, r = u // 2, u % 2
                    nc.tensor.matmul(
                        ps2[64 * r:64 * r + 64, t % 2, qq * W:(qq + 1) * W],
                        w2c, h1[:, u, :], start=True, stop=True,
                        tile_position=(0, 64 * r), skip_group_check=True)
                if t % 2 == 1:
                    h2 = work.tile([128, 2, 4, W], bf16, tag="h2")
                    src2 = ps2[:, :, 0:4 * W].rearrange("p a (c b) -> p a c b",
                                                        b=W)
                    dst2 = h2
                    if t == 1:
                        nc.vector.tensor_scalar(dst2, src2, b2s[:, 0:1], 0.0,
                                                ALU.add, ALU.max)
                    else:
                        nc.scalar.activation(dst2, src2, AF.Relu,
                                             bias=b2s[:, 0:1])
                    for tt in range(2):
                        j = (t - 1) + tt
                        nc.tensor.matmul(
                            ps3[32 * j:32 * j + 32, 0:4 * W], w3s,
                            h2[:, tt, :, :].rearrange("p a b -> p (a b)"),
                            start=True, stop=True, tile_position=(0, 32 * j),
                            skip_group_check=True)
            # ---- exp over all 32 rows' scores ----
            u_sp = spool.tile([98, 4, W], bf16, tag="usp")
            nc.scalar.activation(
                u_sp, ps3[0:98, 0:4 * W].rearrange("p (a b) -> p a b", b=W),
                AF.Exp, bias=b3t[0:98, 0:1])
            wps = wpsp.tile([128, 4, 128], bf16, tag="wps")
            for qq in range(4):
                nc.tensor.transpose(wps[0:W, qq, 0:98], u_sp[:, qq, 0:W],
                                    ident_bf[0:98, 0:98])
            # evac live transpose columns (rows 32j+r) to SBUF for the PE
            wcol = spool.tile([128, 4, 4, 2], bf16, tag="wcol")
            wv = wps.rearrange("p q (j x) -> p q j x", x=32)
            if ci % 2 == 0:
                nc.vector.tensor_copy(wcol[0:W], wv[0:W, :, :, 0:2])
            else:
                nc.scalar.activation(wcol[0:W], wv[0:W, :, :, 0:2], AF.Copy)
            # ---- weighted sums + U ----
            for slot in range(CHUNK):
                j, qq, r = slot // 8, (slot % 8) // 2, slot % 2
                wc = wcol[0:W, qq, j, r:r + 1]
                nc.tensor.matmul(pso[:, slot:slot + 1], ktm[0:W, slot, :], wc,
                                 start=True, stop=True, skip_group_check=True)
                nc.tensor.matmul(pso[0:1, CHUNK + slot:CHUNK + slot + 1],
                                 mskt[0:W, ci, slot:slot + 1], wc,
                                 start=True, stop=True, skip_group_check=True)
            if ci % 2 == 0:
                nc.vector.tensor_copy(outbuf[:, ci, 0:CHUNK], pso[:, 0:CHUNK])
                nc.vector.tensor_copy(outbuf[0:1, ci, CHUNK:2 * CHUNK],
                                      pso[0:1, CHUNK:2 * CHUNK])
            else:
                nc.scalar.activation(outbuf[:, ci, 0:CHUNK], pso[:, 0:CHUNK],
                                     AF.Copy)
                nc.scalar.activation(outbuf[0:1, ci, CHUNK:2 * CHUNK],
                                     pso[0:1, CHUNK:2 * CHUNK], AF.Copy)
        nc.scalar.dma_start(out_d[:], outbuf.rearrange("p a b -> p (a b)"))
    nc.compile()
    return nc


def _host_prep(querys, keys, W1, b1, W2, b2, W3, b3, mask):
    import ml_dtypes
    bf = ml_dtypes.bfloat16
    f8 = ml_dtypes.float8_e4m3
    q = np.ascontiguousarray(querys[:, 0, :], dtype=np.float32)   # [B, E]
    W1a, W1b, W1c, W1d = W1[0:128], W1[128:256], W1[256:384], W1[384:512]
    W1bc = (W1b - W1c).astype(np.float32)
    qb = q @ (W1a + W1c) + b1                                      # [B, H1]
    # min-norm absorption of qb into the two DoubleRow planes
    A = np.concatenate([W1bc.T, W1d.T], axis=1)                    # [128, 256]
    av = (A.T @ np.linalg.solve(A @ A.T, qb.T)).T                  # [B, 256]
    a_b, v_b = av[:, :128], av[:, 128:]

    counts = mask.sum(axis=1).astype(np.int64)                     # [B]
    assert counts.max() <= 128, f"token count {counts.max()} > 128 unsupported"
    order = np.argsort(counts, kind="stable")                      # ascending
    widths = []
    for ci in range(NCHUNK):
        band = order[ci * NCORES * CHUNK:(ci + 1) * NCORES * CHUNK]
        widths.append(max(8, int(-(-counts[band].max() // 8) * 8)))
    widths = tuple(int(w) for w in widths)

    # row assignment: core c, chunk ci, slot s -> order[ci*256 + c*32 + s]
    assign = order.reshape(NCHUNK, NCORES, CHUNK)

    rhs_arrs = [[] for _ in range(NCORES)]
    ktm_arrs = [[] for _ in range(NCORES)]
    mskt_arr = np.zeros((NCORES, 128, NCHUNK, CHUNK), np.float32)
    for ci in range(NCHUNK):
        W = widths[ci]
        for c in range(NCORES):
            rows = assign[ci, c]                                   # [32]
            Kg = np.zeros((CHUNK, W, E), np.float32)
            for s_i, r_i in enumerate(rows):
                toks = np.nonzero(mask[r_i])[0]
                Kg[s_i, :len(toks)] = keys[r_i, toks]
                mskt_arr[c, :len(toks), ci, s_i] = 1.0
            p0 = Kg + a_b[rows][:, None, :]
            p1 = Kg * q[rows][:, None, :] + v_b[rows][:, None, :]
            rhs = np.stack([p0, p1], axis=1)                       # [32,2,W,E]
            rhs_arrs[c].append(np.ascontiguousarray(
                rhs.transpose(3, 0, 1, 2)).astype(f8))             # [E,32,2,W]
            ktm_arrs[c].append(np.ascontiguousarray(
                Kg.transpose(1, 0, 2)).astype(bf))                 # [W,32,E]

    w1dr = np.ascontiguousarray(
        np.stack([W1bc, W1d], axis=1)).astype(f8)                  # [E,2,H1]
    w3s = np.zeros((2 * H2, 32), bf)
    w3s[0:H2, 0] = W3[:, 0].astype(bf)
    w3s[H2:, 1] = W3[:, 0].astype(bf)
    b2s = np.concatenate([b2, b2]).reshape(2 * H2, 1).astype(np.float32)
    return dict(widths=widths, assign=assign, rhs=rhs_arrs, ktm=ktm_arrs,
                mskt=mskt_arr.astype(bf), w1dr=w1dr,
                w2=W2.astype(bf), w3s=w3s, b2s=b2s,
                b3v=np.asarray(b3, np.float32).reshape(1, 1))


def kernel(querys, keys, W1, b1, W2, b2, W3, b3, mask):
    from concourse.bass_utils import run_bass_kernel_spmd

    querys = np.asarray(querys, dtype=np.float32)
    keys = np.asarray(keys, dtype=np.float32)
    W1 = np.asarray(W1, dtype=np.float32)
    b1 = np.asarray(b1, dtype=np.float32)
    W2 = np.asarray(W2, dtype=np.float32)
    b2 = np.asarray(b2, dtype=np.float32)
    W3 = np.asarray(W3, dtype=np.float32)
    b3 = np.asarray(b3, dtype=np.float32)
    mask = np.asarray(mask)
    hp = _host_prep(querys, keys, W1, b1, W2, b2, W3, b3, mask)

    widths = hp["widths"]
    if widths not in _prog_cache:
        _prog_cache[widths] = _build_program(widths)
    prog = _prog_cache[widths]

    in_maps = []
    for c in range(NCORES):
        m = {f"rhs{ci}": hp["rhs"][c][ci] for ci in range(NCHUNK)}
        m.update({f"ktm{ci}": hp["ktm"][c][ci] for ci in range(NCHUNK)})
        m.update({"mskt": hp["mskt"][c], "w1dr": hp["w1dr"], "w2": hp["w2"],
                  "w3s": hp["w3s"], "b2s": hp["b2s"], "b3v": hp["b3v"]})
        in_maps.append(m)

    res = run_bass_kernel_spmd(prog, in_maps, list(range(NCORES)))
    out = np.empty((B, E), np.float32)
    assign = hp["assign"]
    for c in range(NCORES):
        o = res.results[c]["out"].reshape(E, NCHUNK, 2 * CHUNK)
        for ci in range(NCHUNK):
            sums = o[:, ci, 0:CHUNK]                  # [E, 32]
            U = o[0, ci, CHUNK:2 * CHUNK]             # [32]
            out[assign[ci, c]] = (sums / U[None, :]).T
    return out
